# revision 17
# baseline (speedup 1.0000x reference)
"""Trainium2 Bass kernel for nn_DualChannelTransformer.

Sharding: 8 cores = 4 batches x 2 channels (left/right). Each core runs one
channel's transformer stack; cross-attention K/V activations are exchanged
between channel pairs via pairwise AllGather each layer (partner slice read
back with a partition_id-derived dynamic DMA offset so the program stays
SPMD-uniform). Final pooled means are AllGather'd pairwise and both cores of
a pair compute the small classifier heads redundantly.

Optimizations vs the plain bf16 version:
- All dense projections (Q/K/V/O, FFN w1/w2) run in fp8e4 with
  perf_mode=DoubleRow (K=256 per instruction): weights host-packed *256
  (clears the e4m3 subnormal region), descale folded into PSUM
  evacuation; projection biases enter via DR-packed bias-row matmuls or
  evac scalars.
- Attention core (scoresT -> exp -> fused ctx+denominator via ones
  column on V) keeps bf16 Q/K/V; exp tiles are fp8 (softmax weights are
  near-uniform here so quantization is benign, and the fused denominator
  normalizes the quantized weights exactly).
- The per-layer channel exchange ships x in fp8, is issued on the SP
  queue per S-half as soon as the FFN LayerNorm emits that half, and the
  attention core is split into phase A (scores+exp for key-half 0, all
  heads - ACT-paced work that hides the second exchange chunk) and
  phase B (rest + ctx + O).
- LayerNorm: rstd = exp(-0.5*ln(var+eps)) so the whole kernel uses one
  ACT table set (natural_log_exp_and_others); squares on DVE/Pool.
"""

import os
import sys

import numpy as np

for _p in ("/opt/trn_rl_repo", "/root/.axon_site/_ro/trn_rl_repo"):
    if os.path.isdir(_p) and _p not in sys.path:
        sys.path.insert(0, _p)

import ml_dtypes

import concourse.bass as bass
import concourse.tile as tile
from concourse import bacc, mybir

# Force every activation onto the natural_log_exp_and_others table set (it
# holds exp+ln+identity+relu - everything this kernel uses) so bacc's
# table-load pass emits exactly one load instead of thrashing between
# exp_and_others and natural_log_exp_and_others. Only the pass's view of
# set CONTENTS is patched; set ids keep matching act_info.json.
_orig_gat = bacc.get_activation_tables


def _gat_nl_only(arch):
    t = _orig_gat(arch)
    return {name: (fns if name == "natural_log_exp_and_others" else set())
            for name, fns in t.items()}


bacc.get_activation_tables = _gat_nl_only
from concourse.bass import ds
from concourse.bass_utils import run_bass_kernel_spmd

F32 = mybir.dt.float32
F32R = mybir.dt.float32r
BF16 = mybir.dt.bfloat16
FP8 = mybir.dt.float8e4
AF = mybir.ActivationFunctionType
OP = mybir.AluOpType
DR = mybir.MatmulPerfMode.DoubleRow
BF = ml_dtypes.bfloat16
F8 = ml_dtypes.float8_e4m3

B, S, IN, D, H, LAYERS, F = 4, 1024, 6, 512, 8, 4, 2048
DK = D // H
EPS = 1e-5
NCORES = 8
DC = D // 128   # 4 activation partition chunks
FC = F // 128   # 16
NQ = S // 512   # 2 moving-dim chunks
KT8 = S // 128  # 8 k tiles
WS = 256.0      # fp8 weight pre-scale
IWS = 1.0 / WS
# DVE bit-trick exp: bf16_bits(exp(x)) ~= int16(x * EXA + EXB); one-sided
# ~3% rel err whose common component cancels in the softmax ratio.
EXA = 128.0 / np.log(2.0)
EXB = 127.0 * 128.0 - 128.0 * 0.04303

_CACHE = {}


def _pp(pools):
    """[128,512] proj psum carved from the shared [128,1024] pool."""
    t = pools["pbig"].tile([128, 1024], F32, tag="big")
    return t[:, 0:512]


def _emit_ln(nc, pools, a_t, g_sb, b_sb, ln_i, post_half=None):
    """LayerNorm over D (partition axis, 4 chunks) of a_t [128,4,1024] bf16.

    Emits a bf16 tile (residual stream) and an fp8 mirror (matmul moving
    operand). post_half(nq, x_bf, x_f8) fires after each S-half of both
    outputs is complete (used to launch the channel exchange).
    Returns (x_bf, x_f8).
    """
    act, pbig, xpool, f8pool, consts = (pools["act"], pools["pbig"],
                                        pools["xpool"], pools["f8pool"],
                                        pools["consts"])
    oavg_bf = consts["oavg_bf"]    # [128,128] bf16 = 1/512

    x_bf = xpool.tile([128, DC, 1024], BF16, tag="x")
    x_f8 = f8pool.tile([128, DC, 1024], FP8, tag="xf8", bufs=2)
    sq = act.tile([128, 8, 512], BF16, tag="sq", bufs=1)
    for nq in range(NQ):
        s0 = nq * 512
        for kc in range(DC):
            eng = nc.vector if kc != 3 else nc.gpsimd
            eng.tensor_tensor(out=sq[:, kc * NQ + nq, :],
                              in0=a_t[:, kc, s0:s0 + 512],
                              in1=a_t[:, kc, s0:s0 + 512], op=OP.mult)
        mps = pbig.tile([128, 1024], F32, tag="big")
        for kc in range(DC):
            nc.tensor.matmul(mps[:, 0:512], oavg_bf[:],
                             a_t[:, kc, s0:s0 + 512],
                             start=(kc == 0), stop=(kc == DC - 1))
        for kc in range(DC):
            nc.tensor.matmul(mps[:, 512:1024], oavg_bf[:],
                             sq[:, kc * NQ + nq, :],
                             start=(kc == 0), stop=(kc == DC - 1))
        mean_sb = act.tile([128, 512], BF16, tag="mnb", bufs=2)
        nc.scalar.activation(out=mean_sb[:], in_=mps[:, 0:512],
                             func=AF.Identity,
                             bias=consts["zero_col"][:])
        m2 = act.tile([128, 512], BF16, tag="lnt", bufs=2)
        nc.gpsimd.tensor_tensor(out=m2[:], in0=mean_sb[:], in1=mean_sb[:],
                                op=OP.mult)
        work = act.tile([128, 512], F32, tag="lnt2", bufs=2)
        nc.vector.tensor_tensor(out=work[:], in0=mps[:, 512:1024], in1=m2[:],
                                op=OP.subtract)
        # rstd = exp(-0.5 * ln(var + eps)): stays in the exp/ln table set.
        nc.scalar.activation(out=work[:], in_=work[:], func=AF.Ln,
                             bias=consts["eps_col"][:])
        rstd = act.tile([128, 512], BF16, tag="rstd", bufs=2)
        nc.scalar.activation(out=rstd[:], in_=work[:], func=AF.Exp,
                             scale=-0.5)
        for dc in range(DC):
            eng = nc.vector if dc != 3 else nc.gpsimd
            oth = nc.gpsimd
            rg = act.tile([128, 512], BF16, tag="rg", bufs=2)
            nc.vector.tensor_scalar_mul(out=rg[:], in0=rstd[:],
                                        scalar1=g_sb[:, ln_i, dc:dc + 1])
            am = act.tile([128, 512], BF16, tag="am", bufs=2)
            eng.tensor_tensor(out=am[:], in0=a_t[:, dc, s0:s0 + 512],
                              in1=mean_sb[:], op=OP.subtract)
            eng.tensor_tensor(out=am[:], in0=am[:], in1=rg[:], op=OP.mult)
            eng.tensor_scalar_add(out=x_bf[:, dc, s0:s0 + 512], in0=am[:],
                                  scalar1=b_sb[:, ln_i, dc:dc + 1])
            oth.tensor_copy(out=x_f8[:, dc, s0:s0 + 512],
                            in_=x_bf[:, dc, s0:s0 + 512])
        if post_half is not None:
            post_half(nq, x_bf, x_f8)
    return x_bf, x_f8


def _emit_attn(nc, pools, dram, xq_bf, xq_f8, kv_f8, li, bi):
    """One attention block. xq_bf/xq_f8: [128,4,1024] tiles; kv_f8 the K/V
    source (fp8). xq_bf doubles as the residual input. Returns (x_bf, x_f8).
    """
    act, pbig, pctx, consts = (pools["act"], pools["pbig"],
                               pools["pctx"], pools["consts"])
    ones2 = consts["ones2"]      # [1,2,512] fp8 ones (DR bias-row moving)

    wqkv = pools["w"].tile([128, 16, 512], FP8, tag="wbig")
    nc.sync.dma_start(wqkv[:], dram["qkv_w"][li, bi])
    qkb = act.tile([128, 2, 4], F32, tag="qkb", bufs=1)
    nc.sync.dma_start(qkb[:], dram["qk_b"][li, bi])
    vb = act.tile([1, 512], BF16, tag="vb", bufs=1)
    nc.sync.dma_start(vb[:], dram["v_b"][li, bi])
    ob = act.tile([1, 2, 512], FP8, tag="ob", bufs=1)
    nc.sync.dma_start(ob[:], dram["o_b"][li, bi])
    vbb = act.tile([128, 512], BF16, tag="vbb", bufs=1)
    nc.gpsimd.partition_broadcast(out_ap=vbb[:], in_ap=vb[:])

    # ---- Q/K projections (transposed layout, fp8 DoubleRow) ----
    # Order: Q(nq0), Q(nq1), K(nq0), V(st0-3), then phase A of the
    # attention core, then K(nq1), V(st4-7) - so the partner's second
    # exchange chunk is hidden behind phase A.
    QT = act.tile([128, DC, 1024], BF16, tag="qt")
    KT = act.tile([128, DC, 1024], BF16, tag="kt")

    def proj_qk(pi, dst, src, nq):
        s0 = nq * 512
        for mc in range(DC):
            ps = _pp(pools)
            for kg in range(2):
                nc.tensor.matmul(
                    ps,
                    wqkv[:, pi * 4 + 2 * kg:pi * 4 + 2 * kg + 2,
                         mc * 128:(mc + 1) * 128],
                    src[:, 2 * kg:2 * kg + 2, s0:s0 + 512],
                    start=(kg == 0), stop=(kg == 1), perf_mode=DR)
            if mc % 2 == 0:
                nc.vector.tensor_scalar(out=dst[:, mc, s0:s0 + 512],
                                        in0=ps, scalar1=IWS,
                                        scalar2=qkb[:, pi, mc:mc + 1],
                                        op0=OP.mult, op1=OP.add)
            else:
                nc.scalar.activation(out=dst[:, mc, s0:s0 + 512], in_=ps,
                                     func=AF.Identity,
                                     bias=qkb[:, pi, mc:mc + 1], scale=IWS)

    vext = pools["vts"]

    def proj_v(st):
        ps = _pp(pools)
        for kg in range(2):
            nc.tensor.matmul(ps,
                             kv_f8[:, 2 * kg:2 * kg + 2,
                                   st * 128:(st + 1) * 128],
                             wqkv[:, 8 + 2 * kg:8 + 2 * kg + 2, :],
                             start=(kg == 0), stop=(kg == 1), perf_mode=DR)
        nc.vector.scalar_tensor_tensor(
            out=vext[st][:, :, 0:DK],
            in0=ps.rearrange("p (h k) -> p h k", h=H), scalar=IWS,
            in1=vbb[:].rearrange("p (h k) -> p h k", h=H),
            op0=OP.mult, op1=OP.add)

    for nq in range(NQ):
        proj_qk(0, QT, xq_f8, nq)
    proj_qk(1, KT, kv_f8, 0)
    for st in range(4):
        proj_v(st)
    if bi != 0:
        proj_qk(1, KT, kv_f8, 1)
        for st in range(4, KT8):
            proj_v(st)

    # ---- attention core ----
    # Cross blocks (bi=0) split into phase A (scores+exp st0-3, all heads,
    # ACT only - hides the partner's second exchange chunk) and phase B.
    # Self blocks run a single pass per head with exp split ACT/DVE
    # (DVE computes exp via the int16 bit trick writing bf16 bits).
    scale = float(1.0 / np.sqrt(DK))
    phased = bi == 0

    def emit_scores(sps, h, st):
        hp, hr = h // 2, (h % 2) * 64
        for nq in range(NQ):
            nc.tensor.matmul(sps[:, nq * 512:(nq + 1) * 512],
                             KT[hr:hr + 64, hp, st * 128:(st + 1) * 128],
                             QT[hr:hr + 64, hp, nq * 512:(nq + 1) * 512],
                             start=True, stop=True)

    def emit_exp(sps, on_act, out_ap=None):
        if on_act:
            if out_ap is None:
                e = act.tile([128, 1024], FP8, tag="expa", bufs=3)
                out_ap = e[:]
            nc.scalar.activation(out=out_ap, in_=sps[:], func=AF.Exp,
                                 scale=scale)
        else:
            assert out_ap is None
            e = act.tile([128, 1024], BF16, tag="expd", bufs=2)
            out_ap = e[:]
            nc.vector.tensor_scalar(out=e[:].bitcast(mybir.dt.int16),
                                    in0=sps[:], scalar1=float(scale * EXA),
                                    scalar2=float(EXB), op0=OP.mult,
                                    op1=OP.add)
        return out_ap

    def emit_div(cps, ctxT, h):
        hp, hr = h // 2, (h % 2) * 64
        rdb = act.tile([64, 1024], BF16, tag="rdb", bufs=2)
        nc.vector.reciprocal(out=rdb[:], in_=cps[DK:2 * DK, :])
        nc.vector.tensor_tensor(out=ctxT[hr:hr + 64, hp, :], in0=cps[0:DK, :],
                                in1=rdb[:], op=OP.mult)

    ctxT = pools["f8pool"].tile([128, DC, 1024], FP8, tag="ctxf8", bufs=1)
    if phased:
        expA = {}
        early_cps = {}
        for h in range(H):
            eA = pools["vext"].tile([128, 4, 1024], FP8, tag=f"eA{h}")
            for st in range(4):
                sps = pbig.tile([128, 1024], F32, tag="big")
                emit_scores(sps, h, st)
                emit_exp(sps, True, out_ap=eA[:, st, :])
            expA[h] = eA

        for h in range(2):
            cps = pctx.tile([128, 1024], F32, tag="ctx")
            early_cps[h] = cps
            for st0 in range(4):
                for nq in range(NQ):
                    nc.tensor.matmul(cps[:, nq * 512:(nq + 1) * 512],
                                     vext[st0][:, h, :],
                                     expA[h][:, st0, nq * 512:(nq + 1) * 512],
                                     start=(st0 == 0), stop=False)
        proj_qk(1, KT, kv_f8, 1)
        for st in range(4, KT8):
            proj_v(st)

        def emit_ctx(cps, h, st, e_ap, start, stop):
            for nq in range(NQ):
                nc.tensor.matmul(cps[:, nq * 512:(nq + 1) * 512],
                                 vext[st][:, h, :],
                                 e_ap[:, nq * 512:(nq + 1) * 512],
                                 start=start, stop=stop)

        for h in range(H):
            if h in early_cps:
                cps = early_cps[h]
            else:
                cps = pctx.tile([128, 1024], F32, tag="ctx")
            pend = None  # (st, exp_ap) awaiting its ctx matmul
            for st in range(4, KT8):
                sps = pbig.tile([128, 1024], F32, tag="big")
                emit_scores(sps, h, st)
                e = emit_exp(sps, st != 5)
                if st == 4 and h not in early_cps:
                    for st0 in range(4):
                        emit_ctx(cps, h, st0, expA[h][:, st0, :],
                                 st0 == 0, False)
                else:
                    if pend is not None:
                        emit_ctx(cps, h, pend[0], pend[1], False, False)
                pend = (st, e)
            emit_ctx(cps, h, pend[0], pend[1], False, True)
            emit_div(cps, ctxT, h)
    else:
        DVE_ST = (2, 5)  # 2 of 8 exps per head on DVE

        def emit_ctx(cps, h, st, e_ap, start, stop):
            for nq in range(NQ):
                nc.tensor.matmul(cps[:, nq * 512:(nq + 1) * 512],
                                 vext[st][:, h, :],
                                 e_ap[:, nq * 512:(nq + 1) * 512],
                                 start=start, stop=stop)

        for h in range(H):
            cps = pctx.tile([128, 1024], F32, tag="ctx")
            pend = None
            for st in range(KT8):
                sps = pbig.tile([128, 1024], F32, tag="big")
                emit_scores(sps, h, st)
                e = emit_exp(sps, st not in DVE_ST)
                if pend is not None:
                    emit_ctx(cps, h, pend[0], pend[1], pend[0] == 0, False)
                pend = (st, e)
            emit_ctx(cps, h, pend[0], pend[1], False, True)
            emit_div(cps, ctxT, h)

    # ---- O projection + bias(DR row matmul) + residual ----
    a_t = act.tile([128, DC, 1024], BF16, tag="a", bufs=1)
    for nq in range(NQ):
        s0 = nq * 512
        for mc in range(DC):
            ps = _pp(pools)
            nc.tensor.matmul(ps, ob[:, :, mc * 128:(mc + 1) * 128],
                             ones2[:], start=True, stop=False, perf_mode=DR)
            for kg in range(2):
                nc.tensor.matmul(ps,
                                 wqkv[:, 12 + 2 * kg:12 + 2 * kg + 2,
                                      mc * 128:(mc + 1) * 128],
                                 ctxT[:, 2 * kg:2 * kg + 2, s0:s0 + 512],
                                 start=False, stop=(kg == 1), perf_mode=DR)
            nc.vector.scalar_tensor_tensor(out=a_t[:, mc, s0:s0 + 512],
                                           in0=ps, scalar=IWS,
                                           in1=xq_bf[:, mc, s0:s0 + 512],
                                           op0=OP.mult, op1=OP.add)
    return _emit_ln(nc, pools, a_t, pools["lng_sb"][li], pools["lnb_sb"][li],
                    bi)


def _emit_ffn(nc, pools, dram, x_bf, x_f8, li, post_half=None):
    act = pools["act"]
    ones2 = pools["consts"]["ones2"]
    w1 = pools["w"].tile([128, 4, 2048], FP8, tag="wbig")
    nc.sync.dma_start(w1[:], dram["ff_w1"][li])
    w2 = pools["w"].tile([128, 16, 512], FP8, tag="wbig")
    nc.sync.dma_start(w2[:], dram["ff_w2"][li])
    b1r = act.tile([1, 2, 2048], FP8, tag="b1r", bufs=1)
    nc.sync.dma_start(b1r[:], dram["ff_b1r"][li])
    b2r = act.tile([1, 2, 512], FP8, tag="b2r", bufs=1)
    nc.sync.dma_start(b2r[:], dram["ff_b2r"][li])

    a_t = act.tile([128, DC, 1024], BF16, tag="a", bufs=1)
    for half in range(2):
        s0 = half * 512
        hT = pools["f8pool"].tile([128, 16, 512], FP8, tag="hf8", bufs=1)
        for mf in range(FC):
            ps = _pp(pools)
            nc.tensor.matmul(ps, b1r[:, :, mf * 128:(mf + 1) * 128],
                             ones2[:], start=True, stop=False, perf_mode=DR)
            for kg in range(2):
                nc.tensor.matmul(ps,
                                 w1[:, 2 * kg:2 * kg + 2,
                                    mf * 128:(mf + 1) * 128],
                                 x_f8[:, 2 * kg:2 * kg + 2, s0:s0 + 512],
                                 start=False, stop=(kg == 1), perf_mode=DR)
            if mf % 2 == 0:
                nc.vector.tensor_scalar(out=hT[:, mf, :], in0=ps,
                                        scalar1=IWS, scalar2=0.0,
                                        op0=OP.mult, op1=OP.max)
            else:
                nc.scalar.activation(out=hT[:, mf, :], in_=ps,
                                     func=AF.Relu,
                                     bias=pools["consts"]["zero_col"][:],
                                     scale=IWS)
        for mc in range(DC):
            ps = _pp(pools)
            nc.tensor.matmul(ps, b2r[:, :, mc * 128:(mc + 1) * 128],
                             ones2[:], start=True, stop=False, perf_mode=DR)
            for kg in range(FC // 2):
                nc.tensor.matmul(ps,
                                 w2[:, 2 * kg:2 * kg + 2,
                                    mc * 128:(mc + 1) * 128],
                                 hT[:, 2 * kg:2 * kg + 2, :],
                                 start=False, stop=(kg == FC // 2 - 1),
                                 perf_mode=DR)
            nc.vector.scalar_tensor_tensor(out=a_t[:, mc, s0:s0 + 512],
                                           in0=ps, scalar=IWS,
                                           in1=x_bf[:, mc, s0:s0 + 512],
                                           op0=OP.mult, op1=OP.add)
    return _emit_ln(nc, pools, a_t, pools["lng_sb"][li], pools["lnb_sb"][li],
                    2, post_half=post_half)


def _build(n_layers=LAYERS):
    nc = bacc.Bacc("TRN2", target_bir_lowering=False, debug=False,
                   num_devices=NCORES)

    dram = {}
    dram["wT"] = nc.dram_tensor("wT", [IN, S], F32R, kind="ExternalInput")
    dram["w_in"] = nc.dram_tensor("w_in", [IN, D], F32R, kind="ExternalInput")
    dram["b_in"] = nc.dram_tensor("b_in", [128, DC], F32, kind="ExternalInput")
    dram["peT"] = nc.dram_tensor("peT", [128, DC, S], BF16,
                                 kind="ExternalInput")
    dram["qkv_w"] = nc.dram_tensor("qkv_w", [LAYERS, 2, 128, 16, 512], FP8,
                                   kind="ExternalInput")
    dram["qk_b"] = nc.dram_tensor("qk_b", [LAYERS, 2, 128, 2, 4], F32,
                                  kind="ExternalInput")
    dram["v_b"] = nc.dram_tensor("v_b", [LAYERS, 2, 1, 512], BF16,
                                 kind="ExternalInput")
    dram["o_b"] = nc.dram_tensor("o_b", [LAYERS, 2, 1, 2, 512], FP8,
                                 kind="ExternalInput")
    dram["ln_g"] = nc.dram_tensor("ln_g", [128, LAYERS, 3, 4], F32,
                                  kind="ExternalInput")
    dram["ln_b"] = nc.dram_tensor("ln_b", [128, LAYERS, 3, 4], F32,
                                  kind="ExternalInput")
    dram["ff_w1"] = nc.dram_tensor("ff_w1", [LAYERS, 128, 4, 2048], FP8,
                                   kind="ExternalInput")
    dram["ff_b1r"] = nc.dram_tensor("ff_b1r", [LAYERS, 1, 2, 2048], FP8,
                                    kind="ExternalInput")
    dram["ff_w2"] = nc.dram_tensor("ff_w2", [LAYERS, 128, 16, 512], FP8,
                                   kind="ExternalInput")
    dram["ff_b2r"] = nc.dram_tensor("ff_b2r", [LAYERS, 1, 2, 512], FP8,
                                    kind="ExternalInput")
    dram["hd_w1"] = nc.dram_tensor("hd_w1", [2, 128, 8, 512], BF16,
                                   kind="ExternalInput")
    dram["hd_b1"] = nc.dram_tensor("hd_b1", [2, 128, 4], F32,
                                   kind="ExternalInput")
    dram["hd_w2"] = nc.dram_tensor("hd_w2", [2, 128, 4, 2], F32,
                                   kind="ExternalInput")
    dram["hd_b2"] = nc.dram_tensor("hd_b2", [1, 2, 2], F32,
                                   kind="ExternalInput")
    out_logits = nc.dram_tensor("logits", [1, 4], F32, kind="ExternalOutput")

    rg_pairs = [[0, 1], [2, 3], [4, 5], [6, 7]]

    with tile.TileContext(nc) as tc:
        with (
            nc.allow_low_precision(
                reason="deliberate fp8/bf16 activation pipeline"),
            tc.tile_pool(name="act", bufs=1) as act,
            tc.tile_pool(name="w", bufs=3) as wpool,
            tc.tile_pool(name="vext", bufs=1) as vpool,
            tc.tile_pool(name="consts", bufs=1) as cpool,
            tc.tile_pool(name="x", bufs=3) as xpool,
            tc.tile_pool(name="f8", bufs=3) as f8pool,
            tc.tile_pool(name="pbig", bufs=2, space="PSUM") as pbig,
            tc.tile_pool(name="pctx", bufs=2, space="PSUM") as pctx,
            tc.tile_pool(name="dram", bufs=1, space="DRAM") as dpool,
        ):
            # ---- constants ----
            ones2 = cpool.tile([1, 2, 512], FP8, tag="ones2")
            nc.vector.memset(ones2[:], 1.0)
            eps_col = cpool.tile([128, 1], F32, tag="eps_col")
            nc.vector.memset(eps_col[:], EPS)
            zero_col = cpool.tile([128, 1], F32, tag="zero_col")
            nc.vector.memset(zero_col[:], 0.0)
            oavg_bf = cpool.tile([128, 128], BF16, tag="oavg_bf")
            nc.vector.memset(oavg_bf[:], 1.0 / D)
            lng_sb = cpool.tile([128, LAYERS, 3, 4], F32, tag="lng")
            nc.sync.dma_start(lng_sb[:], dram["ln_g"][:])
            lnb_sb = cpool.tile([128, LAYERS, 3, 4], F32, tag="lnb")
            nc.sync.dma_start(lnb_sb[:], dram["ln_b"][:])
            consts = dict(ones2=ones2, oavg_bf=oavg_bf, eps_col=eps_col,
                          zero_col=zero_col)
            pools = dict(act=act, w=wpool, vext=vpool, consts=consts,
                         pbig=pbig, pctx=pctx, xpool=xpool,
                         f8pool=f8pool,
                         lng_sb=[lng_sb[:, li] for li in range(LAYERS)],
                         lnb_sb=[lnb_sb[:, li] for li in range(LAYERS)])

            vts = []
            for st in range(KT8):
                vt = vpool.tile([128, H, DK + 64], BF16, tag=f"v{st}")
                nc.vector.memset(vt[:, :, DK:DK + 64], 1.0)
                vts.append(vt)
            pools["vts"] = vts

            pid = nc.sync.partition_id()
            partner_par = 1 - (pid % 2)

            def start_exchange(li, nq, x_f8, x_part):
                """Stage + AllGather + readback for S-half nq; issued on the
                SP queue so it fires on data-readiness, not engine order."""
                s0 = nq * 512
                ag_in = dpool.tile([128, DC, 512], FP8, tag=f"agi{li}_{nq}")
                ag_out = dpool.tile([2, 128, DC, 512], FP8,
                                    tag=f"ago{li}_{nq}")
                nc.sync.dma_start(ag_in[:], x_f8[:, :, s0:s0 + 512])
                nc.gpsimd.collective_compute(
                    "AllGather", OP.bypass, replica_groups=rg_pairs,
                    ins=[ag_in.opt()], outs=[ag_out.opt()])
                nc.sync.dma_start(
                    x_part[:, :, s0:s0 + 512],
                    ag_out[ds(partner_par, 1), :, :, :].opt())

            # ---- layer 0 input projection: x0T = w_in^T @ wristT + b + peT
            peT_sb = xpool.tile([128, DC, S], BF16, tag="x")
            nc.sync.dma_start(peT_sb[:], dram["peT"][:])
            wT_sb = act.tile([IN, S], F32R, tag="wT")
            nc.sync.dma_start(wT_sb[:], dram["wT"][:])
            win_sb = act.tile([IN, D], F32R, tag="win")
            nc.sync.dma_start(win_sb[:], dram["w_in"][:])
            bin_sb = act.tile([128, DC], F32, tag="bin")
            nc.sync.dma_start(bin_sb[:], dram["b_in"][:])

            x_bf = xpool.tile([128, DC, 1024], BF16, tag="x")
            x_f8 = f8pool.tile([128, DC, 1024], FP8, tag="xf8", bufs=2)
            x_part = f8pool.tile([128, DC, 1024], FP8, tag="xpart", bufs=1)
            for nq in range(NQ):
                s0 = nq * 512
                for mc in range(DC):
                    ps = _pp(pools)
                    nc.tensor.matmul(ps,
                                     win_sb[:, mc * 128:(mc + 1) * 128],
                                     wT_sb[:, s0:s0 + 512],
                                     start=True, stop=True)
                    nc.vector.scalar_tensor_tensor(
                        out=x_bf[:, mc, s0:s0 + 512], in0=ps,
                        scalar=bin_sb[:, mc:mc + 1],
                        in1=peT_sb[:, mc, s0:s0 + 512],
                        op0=OP.add, op1=OP.add)
                    eng = nc.gpsimd if mc % 2 == 0 else nc.vector
                    eng.tensor_copy(out=x_f8[:, mc, s0:s0 + 512],
                                    in_=x_bf[:, mc, s0:s0 + 512])
                start_exchange(0, nq, x_f8, x_part)

            for li in range(n_layers):
                lw = li % LAYERS
                xc_bf, xc_f8 = _emit_attn(nc, pools, dram, x_bf, x_f8,
                                          x_part, lw, 0)
                xs_bf, xs_f8 = _emit_attn(nc, pools, dram, xc_bf, xc_f8,
                                          xc_f8, lw, 1)
                last = li == n_layers - 1
                post_half = None
                if not last:
                    x_part = f8pool.tile([128, DC, 1024], FP8, tag="xpart",
                                         bufs=1)

                    def post_half(nq, xbf_new, xf8_new, _li=li + 1,
                                  _xp=x_part):
                        start_exchange(_li, nq, xf8_new, _xp)

                x_bf, x_f8 = _emit_ffn(nc, pools, dram, xs_bf, xs_f8, lw,
                                       post_half=post_half)

            # ---- mean pool over S -> pairwise allgather -> heads ----
            mh = act.tile([128, DC, 2], F32, tag="meanh")
            for nq in range(NQ):
                for dc in range(DC):
                    nc.vector.tensor_reduce(out=mh[:, dc, nq:nq + 1],
                                            in_=x_bf[:, dc,
                                                     nq * 512:(nq + 1) * 512],
                                            axis=mybir.AxisListType.X,
                                            op=OP.add)
            mean_sb = act.tile([128, DC, 1], F32, tag="mean")
            for dc in range(DC):
                nc.vector.tensor_tensor(out=mean_sb[:, dc, :],
                                        in0=mh[:, dc, 0:1],
                                        in1=mh[:, dc, 1:2], op=OP.add)
            mb_in = dpool.tile([DC, 128, 1], F32, tag="mbin")
            for dc in range(DC):
                nc.sync.dma_start(mb_in[dc], mean_sb[:, dc, :])
            mb_out = dpool.tile([2 * DC, 128, 1], F32, tag="mbout")
            nc.gpsimd.collective_compute(
                "AllGather", OP.bypass, replica_groups=rg_pairs,
                ins=[mb_in.opt()], outs=[mb_out.opt()])
            fusedF = act.tile([128, 2 * DC, 1], F32, tag="fusedF")
            for kc in range(2 * DC):
                nc.sync.dma_start(fusedF[:, kc, :], mb_out[kc])
            fusedT = act.tile([128, 2 * DC, 1], BF16, tag="fusedT")
            nc.vector.tensor_copy(out=fusedT[:], in_=fusedF[:])

            hb2 = act.tile([1, 2, 2], F32, tag="hb2")
            nc.sync.dma_start(hb2[:], dram["hd_b2"][:])
            logits_sb = act.tile([1, 4], F32, tag="logits")
            for hd in range(2):
                hw1 = act.tile([128, 8, 512], BF16, tag="hT", bufs=1)
                nc.sync.dma_start(hw1[:], dram["hd_w1"][hd])
                hw2 = act.tile([128, 4, 2], F32, tag="hw2", bufs=2)
                nc.sync.dma_start(hw2[:], dram["hd_w2"][hd])
                hb1 = act.tile([128, 4], F32, tag="hb1", bufs=2)
                nc.sync.dma_start(hb1[:], dram["hd_b1"][hd])
                o1 = act.tile([128, 4, 1], F32, tag="o1", bufs=2)
                for mc in range(DC):
                    ps = _pp(pools)
                    for kc in range(2 * DC):
                        nc.tensor.matmul(
                            ps[:, 0:1],
                            hw1[:, kc, mc * 128:(mc + 1) * 128],
                            fusedT[:, kc, :],
                            start=(kc == 0), stop=(kc == 2 * DC - 1))
                    nc.vector.tensor_scalar(out=o1[:, mc, :], in0=ps[:, 0:1],
                                            scalar1=hb1[:, mc:mc + 1],
                                            scalar2=0.0, op0=OP.add,
                                            op1=OP.max)
                lp = _pp(pools)
                for kc in range(DC):
                    nc.tensor.matmul(lp[0:1, 0:2], o1[:, kc, :],
                                     hw2[:, kc, :],
                                     start=(kc == 0), stop=(kc == DC - 1))
                nc.vector.tensor_tensor(out=logits_sb[0:1, hd * 2:hd * 2 + 2],
                                        in0=lp[0:1, 0:2], in1=hb2[0:1, hd, :],
                                        op=OP.add)
            nc.sync.dma_start(out_logits[:], logits_sb[:])

    nc.compile()
    return nc


def _prep(inputs):
    f32 = np.float32

    def g(k):
        return np.asarray(inputs[k], f32)

    lw, rw = g("left_wrist"), g("right_wrist")
    Wl, bl, Wr, br, pe = g("Wl"), g("bl"), g("Wr"), g("br"), g("pe")
    mha_w, mha_b = g("mha_w"), g("mha_b")
    mha_ln_g, mha_ln_b = g("mha_ln_g"), g("mha_ln_b")
    ff_w1, ff_b1, ff_w2, ff_b2 = g("ff_w1"), g("ff_b1"), g("ff_w2"), g("ff_b2")
    ff_ln_g, ff_ln_b = g("ff_ln_g"), g("ff_ln_b")
    h_w1 = [g("h1_w1"), g("h2_w1")]
    h_b1 = [g("h1_b1"), g("h2_b1")]
    h_w2 = [g("h1_w2"), g("h2_w2")]
    h_b2 = [g("h1_b2"), g("h2_b2")]

    peT = np.ascontiguousarray(
        pe.T.reshape(DC, 128, S).transpose(1, 0, 2)).astype(BF)

    per_ch = {}
    for ch in range(2):
        blocks = (0, 2) if ch == 0 else (1, 3)
        qkv = np.zeros((LAYERS, 2, 128, 16, 512), F8)
        qkb = np.zeros((LAYERS, 2, 128, 2, 4), f32)
        vb = np.zeros((LAYERS, 2, 1, 512), BF)
        obr = np.zeros((LAYERS, 2, 1, 2, 512), F8)
        lng = np.zeros((128, LAYERS, 3, 4), f32)
        lnb = np.zeros((128, LAYERS, 3, 4), f32)
        fw1 = np.zeros((LAYERS, 128, 4, 2048), F8)
        fb1r = np.zeros((LAYERS, 1, 2, 2048), F8)
        fw2 = np.zeros((LAYERS, 128, 16, 512), F8)
        fb2r = np.zeros((LAYERS, 1, 2, 512), F8)
        for li in range(LAYERS):
            for bi, blk in enumerate(blocks):
                for pi in range(3):  # q, k, v
                    qkv[li, bi, :, pi * 4:(pi + 1) * 4, :] = \
                        (mha_w[li, blk, pi] * WS).reshape(DC, 128, D) \
                        .transpose(1, 0, 2).astype(F8)
                qkv[li, bi, :, 12:16, :] = (mha_w[li, blk, 3] * WS) \
                    .reshape(DC, 128, D).transpose(1, 0, 2).astype(F8)
                for ci, pi in enumerate((0, 1)):  # q, k biases (unscaled)
                    qkb[li, bi, :, ci, :] = \
                        mha_b[li, blk, pi].reshape(DC, 128).T
                vb[li, bi, 0] = mha_b[li, blk, 2].astype(BF)
                obr[li, bi, 0, 0] = (mha_b[li, blk, 3] * WS).astype(F8)
                lng[:, li, bi, :] = mha_ln_g[li, blk].reshape(DC, 128).T
                lnb[:, li, bi, :] = mha_ln_b[li, blk].reshape(DC, 128).T
            lng[:, li, 2, :] = ff_ln_g[li, ch].reshape(DC, 128).T
            lnb[:, li, 2, :] = ff_ln_b[li, ch].reshape(DC, 128).T
            fw1[li] = (ff_w1[li, ch] * WS).reshape(DC, 128, F) \
                .transpose(1, 0, 2).astype(F8)
            fb1r[li, 0, 0] = (ff_b1[li, ch] * WS).astype(F8)
            fw2[li] = (ff_w2[li, ch] * WS).reshape(FC, 128, D) \
                .transpose(1, 0, 2).astype(F8)
            fb2r[li, 0, 0] = (ff_b2[li, ch] * WS).astype(F8)
        per_ch[ch] = dict(qkv_w=qkv, qk_b=qkb, v_b=vb, o_b=obr,
                          ln_g=lng, ln_b=lnb, ff_w1=fw1, ff_b1r=fb1r,
                          ff_w2=fw2, ff_b2r=fb2r)

    hd_w1 = np.stack([(w / float(S)).reshape(2 * DC, 128, D)
                      .transpose(1, 0, 2) for w in h_w1]).astype(BF)
    hd_b1 = np.stack([b.reshape(DC, 128).T for b in h_b1]).astype(f32)
    hd_w2 = np.stack([w.reshape(DC, 128, 2).transpose(1, 0, 2)
                      for w in h_w2]).astype(f32)
    hd_b2 = np.stack([b.reshape(1, 2) for b in h_b2]).transpose(1, 0, 2).astype(f32)

    in_maps = []
    for core in range(NCORES):
        b, ch = core // 2, core % 2
        wrist = lw[b] if ch == 0 else rw[b]
        w_in = Wl if ch == 0 else Wr
        b_in = (bl if ch == 0 else br).reshape(DC, 128).T
        m = {k: np.ascontiguousarray(v) for k, v in per_ch[ch].items()}
        m["wT"] = np.ascontiguousarray(wrist.T)
        m["w_in"] = np.ascontiguousarray(w_in)
        m["b_in"] = np.ascontiguousarray(b_in.astype(f32))
        m["peT"] = peT
        m["hd_w1"] = hd_w1
        m["hd_b1"] = hd_b1
        m["hd_w2"] = hd_w2
        m["hd_b2"] = hd_b2
        in_maps.append(m)
    return in_maps


def run(inputs, trace=False, n_layers=LAYERS):
    key = ("nc", n_layers)
    if key not in _CACHE:
        _CACHE[key] = _build(n_layers)
    nc = _CACHE[key]
    in_maps = _prep(inputs)
    res = run_bass_kernel_spmd(nc, in_maps, core_ids=list(range(NCORES)),
                               trace=trace)
    logits1 = np.zeros((B, 2), np.float32)
    logits2 = np.zeros((B, 2), np.float32)
    for b in range(B):
        out = res.results[2 * b]["logits"]
        logits1[b] = out[0, 0:2]
        logits2[b] = out[0, 2:4]
    return (logits1, logits2), res


def kernel(**inputs):
    out, _ = run(inputs, trace=False)
    return out


# revision 18
# speedup vs baseline: 1.0146x; 1.0146x over previous
"""Trainium2 Bass kernel for nn_DualChannelTransformer.

Sharding: 8 cores = 4 batches x 2 channels (left/right). Each core runs one
channel's transformer stack; cross-attention K/V activations are exchanged
between channel pairs via pairwise AllGather each layer (partner slice read
back with a partition_id-derived dynamic DMA offset so the program stays
SPMD-uniform). Final pooled means are AllGather'd pairwise and both cores of
a pair compute the small classifier heads redundantly.

Optimizations vs the plain bf16 version:
- All dense projections (Q/K/V/O, FFN w1/w2) run in fp8e4 with
  perf_mode=DoubleRow (K=256 per instruction): weights host-packed *256
  (clears the e4m3 subnormal region), descale folded into PSUM
  evacuation; projection biases enter via DR-packed bias-row matmuls or
  evac scalars.
- Attention core (scoresT -> exp -> fused ctx+denominator via ones
  column on V) keeps bf16 Q/K/V; exp tiles are fp8 (softmax weights are
  near-uniform here so quantization is benign, and the fused denominator
  normalizes the quantized weights exactly).
- The per-layer channel exchange ships x in fp8, is issued on the SP
  queue per S-half as soon as the FFN LayerNorm emits that half, and the
  attention core is split into phase A (scores+exp for key-half 0, all
  heads - ACT-paced work that hides the second exchange chunk) and
  phase B (rest + ctx + O).
- LayerNorm: rstd = exp(-0.5*ln(var+eps)) so the whole kernel uses one
  ACT table set (natural_log_exp_and_others); squares on DVE/Pool.
"""

import os
import sys

import numpy as np

for _p in ("/opt/trn_rl_repo", "/root/.axon_site/_ro/trn_rl_repo"):
    if os.path.isdir(_p) and _p not in sys.path:
        sys.path.insert(0, _p)

import ml_dtypes

import concourse.bass as bass
import concourse.tile as tile
from concourse import bacc, mybir

# Force every activation onto the natural_log_exp_and_others table set (it
# holds exp+ln+identity+relu - everything this kernel uses) so bacc's
# table-load pass emits exactly one load instead of thrashing between
# exp_and_others and natural_log_exp_and_others. Only the pass's view of
# set CONTENTS is patched; set ids keep matching act_info.json.
_orig_gat = bacc.get_activation_tables


def _gat_nl_only(arch):
    t = _orig_gat(arch)
    return {name: (fns if name == "natural_log_exp_and_others" else set())
            for name, fns in t.items()}


bacc.get_activation_tables = _gat_nl_only
from concourse.bass import ds
from concourse.bass_utils import run_bass_kernel_spmd

F32 = mybir.dt.float32
F32R = mybir.dt.float32r
BF16 = mybir.dt.bfloat16
FP8 = mybir.dt.float8e4
AF = mybir.ActivationFunctionType
OP = mybir.AluOpType
DR = mybir.MatmulPerfMode.DoubleRow
BF = ml_dtypes.bfloat16
F8 = ml_dtypes.float8_e4m3

B, S, IN, D, H, LAYERS, F = 4, 1024, 6, 512, 8, 4, 2048
DK = D // H
EPS = 1e-5
NCORES = 8
DC = D // 128   # 4 activation partition chunks
FC = F // 128   # 16
NQ = S // 512   # 2 moving-dim chunks
KT8 = S // 128  # 8 k tiles
WS = 256.0      # fp8 weight pre-scale
IWS = 1.0 / WS
# DVE bit-trick exp: bf16_bits(exp(x)) ~= int16(x * EXA + EXB); one-sided
# ~3% rel err whose common component cancels in the softmax ratio.
EXA = 128.0 / np.log(2.0)
EXB = 127.0 * 128.0 - 128.0 * 0.04303

_CACHE = {}


def _pp(pools):
    """[128,512] proj psum carved from the shared [128,1024] pool."""
    t = pools["pbig"].tile([128, 1024], F32, tag="big")
    return t[:, 0:512]


def _emit_ln(nc, pools, a_t, g_sb, b_sb, ln_i, post_half=None):
    """LayerNorm over D (partition axis, 4 chunks) of a_t [128,4,1024] bf16.

    Emits a bf16 tile (residual stream) and an fp8 mirror (matmul moving
    operand). post_half(nq, x_bf, x_f8) fires after each S-half of both
    outputs is complete (used to launch the channel exchange).
    Returns (x_bf, x_f8).
    """
    act, pbig, xpool, f8pool, consts = (pools["act"], pools["pbig"],
                                        pools["xpool"], pools["f8pool"],
                                        pools["consts"])
    oavg_bf = consts["oavg_bf"]    # [128,128] bf16 = 1/512

    x_bf = xpool.tile([128, DC, 1024], BF16, tag="x")
    x_f8 = f8pool.tile([128, DC, 1024], FP8, tag="xf8", bufs=2)
    sq = act.tile([128, 8, 512], BF16, tag="sq", bufs=1)
    for nq in range(NQ):
        s0 = nq * 512
        for kc in range(DC):
            eng = nc.vector if kc != 3 else nc.gpsimd
            eng.tensor_tensor(out=sq[:, kc * NQ + nq, :],
                              in0=a_t[:, kc, s0:s0 + 512],
                              in1=a_t[:, kc, s0:s0 + 512], op=OP.mult)
        mps = pbig.tile([128, 1024], F32, tag="big")
        for kc in range(DC):
            nc.tensor.matmul(mps[:, 0:512], oavg_bf[:],
                             a_t[:, kc, s0:s0 + 512],
                             start=(kc == 0), stop=(kc == DC - 1))
        for kc in range(DC):
            nc.tensor.matmul(mps[:, 512:1024], oavg_bf[:],
                             sq[:, kc * NQ + nq, :],
                             start=(kc == 0), stop=(kc == DC - 1))
        mean_sb = act.tile([128, 512], BF16, tag="mnb", bufs=2)
        nc.scalar.activation(out=mean_sb[:], in_=mps[:, 0:512],
                             func=AF.Identity,
                             bias=consts["zero_col"][:])
        m2 = act.tile([128, 512], BF16, tag="lnt", bufs=2)
        nc.vector.tensor_tensor(out=m2[:], in0=mean_sb[:], in1=mean_sb[:],
                                op=OP.mult)
        work = act.tile([128, 512], F32, tag="lnt2", bufs=2)
        nc.vector.tensor_tensor(out=work[:], in0=mps[:, 512:1024], in1=m2[:],
                                op=OP.subtract)
        # rstd = exp(-0.5 * ln(var + eps)): stays in the exp/ln table set.
        nc.scalar.activation(out=work[:], in_=work[:], func=AF.Ln,
                             bias=consts["eps_col"][:])
        rstd = act.tile([128, 512], BF16, tag="rstd", bufs=2)
        nc.scalar.activation(out=rstd[:], in_=work[:], func=AF.Exp,
                             scale=-0.5)
        for dc in range(DC):
            eng = nc.vector if dc != 3 else nc.gpsimd
            oth = nc.gpsimd if dc % 2 == 0 else nc.vector
            rg = act.tile([128, 512], BF16, tag="rg", bufs=2)
            nc.vector.tensor_scalar_mul(out=rg[:], in0=rstd[:],
                                        scalar1=g_sb[:, ln_i, dc:dc + 1])
            am = act.tile([128, 512], BF16, tag="am", bufs=2)
            eng.tensor_tensor(out=am[:], in0=a_t[:, dc, s0:s0 + 512],
                              in1=mean_sb[:], op=OP.subtract)
            eng.tensor_tensor(out=am[:], in0=am[:], in1=rg[:], op=OP.mult)
            eng.tensor_scalar_add(out=x_bf[:, dc, s0:s0 + 512], in0=am[:],
                                  scalar1=b_sb[:, ln_i, dc:dc + 1])
            oth.tensor_copy(out=x_f8[:, dc, s0:s0 + 512],
                            in_=x_bf[:, dc, s0:s0 + 512])
        if post_half is not None:
            post_half(nq, x_bf, x_f8)
    return x_bf, x_f8


def _emit_attn(nc, pools, dram, xq_bf, xq_f8, kv_f8, li, bi):
    """One attention block. xq_bf/xq_f8: [128,4,1024] tiles; kv_f8 the K/V
    source (fp8). xq_bf doubles as the residual input. Returns (x_bf, x_f8).
    """
    act, pbig, pctx, consts = (pools["act"], pools["pbig"],
                               pools["pctx"], pools["consts"])
    ones2 = consts["ones2"]      # [1,2,512] fp8 ones (DR bias-row moving)

    wqkv = pools["w"].tile([128, 16, 512], FP8, tag="wbig")
    nc.sync.dma_start(wqkv[:], dram["qkv_w"][li, bi])
    qkb = act.tile([128, 2, 4], F32, tag="qkb", bufs=1)
    nc.sync.dma_start(qkb[:], dram["qk_b"][li, bi])
    vb = act.tile([1, 512], BF16, tag="vb", bufs=1)
    nc.sync.dma_start(vb[:], dram["v_b"][li, bi])
    ob = act.tile([1, 2, 512], FP8, tag="ob", bufs=1)
    nc.sync.dma_start(ob[:], dram["o_b"][li, bi])
    vbb = act.tile([128, 512], BF16, tag="vbb", bufs=1)
    nc.gpsimd.partition_broadcast(out_ap=vbb[:], in_ap=vb[:])

    # ---- Q/K projections (transposed layout, fp8 DoubleRow) ----
    # Order: Q(nq0), Q(nq1), K(nq0), V(st0-3), then phase A of the
    # attention core, then K(nq1), V(st4-7) - so the partner's second
    # exchange chunk is hidden behind phase A.
    QT = act.tile([128, DC, 1024], BF16, tag="qt")
    KT = act.tile([128, DC, 1024], BF16, tag="kt")

    def proj_qk(pi, dst, src, nq):
        s0 = nq * 512
        for mc in range(DC):
            ps = _pp(pools)
            for kg in range(2):
                nc.tensor.matmul(
                    ps,
                    wqkv[:, pi * 4 + 2 * kg:pi * 4 + 2 * kg + 2,
                         mc * 128:(mc + 1) * 128],
                    src[:, 2 * kg:2 * kg + 2, s0:s0 + 512],
                    start=(kg == 0), stop=(kg == 1), perf_mode=DR)
            if mc % 2 == 0:
                nc.vector.tensor_scalar(out=dst[:, mc, s0:s0 + 512],
                                        in0=ps, scalar1=IWS,
                                        scalar2=qkb[:, pi, mc:mc + 1],
                                        op0=OP.mult, op1=OP.add)
            else:
                nc.scalar.activation(out=dst[:, mc, s0:s0 + 512], in_=ps,
                                     func=AF.Identity,
                                     bias=qkb[:, pi, mc:mc + 1], scale=IWS)

    vext = pools["vts"]

    def proj_v(st):
        ps = _pp(pools)
        for kg in range(2):
            nc.tensor.matmul(ps,
                             kv_f8[:, 2 * kg:2 * kg + 2,
                                   st * 128:(st + 1) * 128],
                             wqkv[:, 8 + 2 * kg:8 + 2 * kg + 2, :],
                             start=(kg == 0), stop=(kg == 1), perf_mode=DR)
        nc.vector.scalar_tensor_tensor(
            out=vext[st][:, :, 0:DK],
            in0=ps.rearrange("p (h k) -> p h k", h=H), scalar=IWS,
            in1=vbb[:].rearrange("p (h k) -> p h k", h=H),
            op0=OP.mult, op1=OP.add)

    for nq in range(NQ):
        proj_qk(0, QT, xq_f8, nq)
    proj_qk(1, KT, kv_f8, 0)
    for st in range(4):
        proj_v(st)
    if bi != 0:
        proj_qk(1, KT, kv_f8, 1)
        for st in range(4, KT8):
            proj_v(st)

    # ---- attention core ----
    # Cross blocks (bi=0) split into phase A (scores+exp st0-3, all heads,
    # ACT only - hides the partner's second exchange chunk) and phase B.
    # Self blocks run a single pass per head with exp split ACT/DVE
    # (DVE computes exp via the int16 bit trick writing bf16 bits).
    scale = float(1.0 / np.sqrt(DK))
    phased = bi == 0

    def emit_scores(sps, h, st):
        hp, hr = h // 2, (h % 2) * 64
        for nq in range(NQ):
            nc.tensor.matmul(sps[:, nq * 512:(nq + 1) * 512],
                             KT[hr:hr + 64, hp, st * 128:(st + 1) * 128],
                             QT[hr:hr + 64, hp, nq * 512:(nq + 1) * 512],
                             start=True, stop=True)

    def emit_exp(sps, on_act, out_ap=None):
        if on_act:
            if out_ap is None:
                e = act.tile([128, 1024], FP8, tag="expa", bufs=3)
                out_ap = e[:]
            nc.scalar.activation(out=out_ap, in_=sps[:], func=AF.Exp,
                                 scale=scale)
        else:
            assert out_ap is None
            e = act.tile([128, 1024], BF16, tag="expd", bufs=2)
            out_ap = e[:]
            nc.vector.tensor_scalar(out=e[:].bitcast(mybir.dt.int16),
                                    in0=sps[:], scalar1=float(scale * EXA),
                                    scalar2=float(EXB), op0=OP.mult,
                                    op1=OP.add)
        return out_ap

    def emit_div(cps, ctxT, h):
        hp, hr = h // 2, (h % 2) * 64
        rdb = act.tile([64, 1024], BF16, tag="rdb", bufs=2)
        nc.vector.reciprocal(out=rdb[:], in_=cps[DK:2 * DK, :])
        nc.vector.tensor_tensor(out=ctxT[hr:hr + 64, hp, :], in0=cps[0:DK, :],
                                in1=rdb[:], op=OP.mult)

    ctxT = pools["f8pool"].tile([128, DC, 1024], FP8, tag="ctxf8", bufs=1)
    if phased:
        expA = {}
        early_cps = {}
        for h in range(H):
            eA = pools["vext"].tile([128, 4, 1024], FP8, tag=f"eA{h}")
            for st in range(4):
                sps = pbig.tile([128, 1024], F32, tag="big")
                emit_scores(sps, h, st)
                emit_exp(sps, True, out_ap=eA[:, st, :])
            expA[h] = eA

        for h in range(2):
            cps = pctx.tile([128, 1024], F32, tag="ctx")
            early_cps[h] = cps
            for st0 in range(4):
                for nq in range(NQ):
                    nc.tensor.matmul(cps[:, nq * 512:(nq + 1) * 512],
                                     vext[st0][:, h, :],
                                     expA[h][:, st0, nq * 512:(nq + 1) * 512],
                                     start=(st0 == 0), stop=False)
        proj_qk(1, KT, kv_f8, 1)
        for st in range(4, KT8):
            proj_v(st)

        def emit_ctx(cps, h, st, e_ap, start, stop):
            for nq in range(NQ):
                nc.tensor.matmul(cps[:, nq * 512:(nq + 1) * 512],
                                 vext[st][:, h, :],
                                 e_ap[:, nq * 512:(nq + 1) * 512],
                                 start=start, stop=stop)

        for h in range(H):
            if h in early_cps:
                cps = early_cps[h]
            else:
                cps = pctx.tile([128, 1024], F32, tag="ctx")
            pend = None  # (st, exp_ap) awaiting its ctx matmul
            for st in range(4, KT8):
                sps = pbig.tile([128, 1024], F32, tag="big")
                emit_scores(sps, h, st)
                e = emit_exp(sps, st != 5)
                if st == 4 and h not in early_cps:
                    for st0 in range(4):
                        emit_ctx(cps, h, st0, expA[h][:, st0, :],
                                 st0 == 0, False)
                else:
                    if pend is not None:
                        emit_ctx(cps, h, pend[0], pend[1], False, False)
                pend = (st, e)
            emit_ctx(cps, h, pend[0], pend[1], False, True)
            emit_div(cps, ctxT, h)
    else:
        DVE_ST = (2, 5)  # 2 of 8 exps per head on DVE

        def emit_ctx(cps, h, st, e_ap, start, stop):
            for nq in range(NQ):
                nc.tensor.matmul(cps[:, nq * 512:(nq + 1) * 512],
                                 vext[st][:, h, :],
                                 e_ap[:, nq * 512:(nq + 1) * 512],
                                 start=start, stop=stop)

        for h in range(H):
            cps = pctx.tile([128, 1024], F32, tag="ctx")
            pend = None
            for st in range(KT8):
                sps = pbig.tile([128, 1024], F32, tag="big")
                emit_scores(sps, h, st)
                e = emit_exp(sps, st not in DVE_ST)
                if pend is not None:
                    emit_ctx(cps, h, pend[0], pend[1], pend[0] == 0, False)
                pend = (st, e)
            emit_ctx(cps, h, pend[0], pend[1], False, True)
            emit_div(cps, ctxT, h)

    # ---- O projection + bias(DR row matmul) + residual ----
    a_t = act.tile([128, DC, 1024], BF16, tag="a", bufs=1)
    for nq in range(NQ):
        s0 = nq * 512
        for mc in range(DC):
            ps = _pp(pools)
            nc.tensor.matmul(ps, ob[:, :, mc * 128:(mc + 1) * 128],
                             ones2[:], start=True, stop=False, perf_mode=DR)
            for kg in range(2):
                nc.tensor.matmul(ps,
                                 wqkv[:, 12 + 2 * kg:12 + 2 * kg + 2,
                                      mc * 128:(mc + 1) * 128],
                                 ctxT[:, 2 * kg:2 * kg + 2, s0:s0 + 512],
                                 start=False, stop=(kg == 1), perf_mode=DR)
            nc.vector.scalar_tensor_tensor(out=a_t[:, mc, s0:s0 + 512],
                                           in0=ps, scalar=IWS,
                                           in1=xq_bf[:, mc, s0:s0 + 512],
                                           op0=OP.mult, op1=OP.add)
    return _emit_ln(nc, pools, a_t, pools["lng_sb"][li], pools["lnb_sb"][li],
                    bi)


def _emit_ffn(nc, pools, dram, x_bf, x_f8, li, post_half=None):
    act = pools["act"]
    ones2 = pools["consts"]["ones2"]
    w1 = pools["w"].tile([128, 4, 2048], FP8, tag="wbig")
    nc.sync.dma_start(w1[:], dram["ff_w1"][li])
    w2 = pools["w"].tile([128, 16, 512], FP8, tag="wbig")
    nc.sync.dma_start(w2[:], dram["ff_w2"][li])
    b1r = act.tile([1, 2, 2048], FP8, tag="b1r", bufs=1)
    nc.sync.dma_start(b1r[:], dram["ff_b1r"][li])
    b2r = act.tile([1, 2, 512], FP8, tag="b2r", bufs=1)
    nc.sync.dma_start(b2r[:], dram["ff_b2r"][li])

    a_t = act.tile([128, DC, 1024], BF16, tag="a", bufs=1)
    for half in range(2):
        s0 = half * 512
        hT = pools["f8pool"].tile([128, 16, 512], FP8, tag="hf8", bufs=1)
        for mf in range(FC):
            ps = _pp(pools)
            nc.tensor.matmul(ps, b1r[:, :, mf * 128:(mf + 1) * 128],
                             ones2[:], start=True, stop=False, perf_mode=DR)
            for kg in range(2):
                nc.tensor.matmul(ps,
                                 w1[:, 2 * kg:2 * kg + 2,
                                    mf * 128:(mf + 1) * 128],
                                 x_f8[:, 2 * kg:2 * kg + 2, s0:s0 + 512],
                                 start=False, stop=(kg == 1), perf_mode=DR)
            if mf % 2 == 0:
                nc.vector.tensor_scalar(out=hT[:, mf, :], in0=ps,
                                        scalar1=IWS, scalar2=0.0,
                                        op0=OP.mult, op1=OP.max)
            else:
                nc.scalar.activation(out=hT[:, mf, :], in_=ps,
                                     func=AF.Relu,
                                     bias=pools["consts"]["zero_col"][:],
                                     scale=IWS)
        for mc in range(DC):
            ps = _pp(pools)
            nc.tensor.matmul(ps, b2r[:, :, mc * 128:(mc + 1) * 128],
                             ones2[:], start=True, stop=False, perf_mode=DR)
            for kg in range(FC // 2):
                nc.tensor.matmul(ps,
                                 w2[:, 2 * kg:2 * kg + 2,
                                    mc * 128:(mc + 1) * 128],
                                 hT[:, 2 * kg:2 * kg + 2, :],
                                 start=False, stop=(kg == FC // 2 - 1),
                                 perf_mode=DR)
            nc.vector.scalar_tensor_tensor(out=a_t[:, mc, s0:s0 + 512],
                                           in0=ps, scalar=IWS,
                                           in1=x_bf[:, mc, s0:s0 + 512],
                                           op0=OP.mult, op1=OP.add)
    return _emit_ln(nc, pools, a_t, pools["lng_sb"][li], pools["lnb_sb"][li],
                    2, post_half=post_half)


def _build(n_layers=LAYERS):
    nc = bacc.Bacc("TRN2", target_bir_lowering=False, debug=False,
                   num_devices=NCORES)

    dram = {}
    dram["wT"] = nc.dram_tensor("wT", [IN, S], F32R, kind="ExternalInput")
    dram["w_in"] = nc.dram_tensor("w_in", [IN, D], F32R, kind="ExternalInput")
    dram["b_in"] = nc.dram_tensor("b_in", [128, DC], F32, kind="ExternalInput")
    dram["peT"] = nc.dram_tensor("peT", [128, DC, S], BF16,
                                 kind="ExternalInput")
    dram["qkv_w"] = nc.dram_tensor("qkv_w", [LAYERS, 2, 128, 16, 512], FP8,
                                   kind="ExternalInput")
    dram["qk_b"] = nc.dram_tensor("qk_b", [LAYERS, 2, 128, 2, 4], F32,
                                  kind="ExternalInput")
    dram["v_b"] = nc.dram_tensor("v_b", [LAYERS, 2, 1, 512], BF16,
                                 kind="ExternalInput")
    dram["o_b"] = nc.dram_tensor("o_b", [LAYERS, 2, 1, 2, 512], FP8,
                                 kind="ExternalInput")
    dram["ln_g"] = nc.dram_tensor("ln_g", [128, LAYERS, 3, 4], F32,
                                  kind="ExternalInput")
    dram["ln_b"] = nc.dram_tensor("ln_b", [128, LAYERS, 3, 4], F32,
                                  kind="ExternalInput")
    dram["ff_w1"] = nc.dram_tensor("ff_w1", [LAYERS, 128, 4, 2048], FP8,
                                   kind="ExternalInput")
    dram["ff_b1r"] = nc.dram_tensor("ff_b1r", [LAYERS, 1, 2, 2048], FP8,
                                    kind="ExternalInput")
    dram["ff_w2"] = nc.dram_tensor("ff_w2", [LAYERS, 128, 16, 512], FP8,
                                   kind="ExternalInput")
    dram["ff_b2r"] = nc.dram_tensor("ff_b2r", [LAYERS, 1, 2, 512], FP8,
                                    kind="ExternalInput")
    dram["hd_w1"] = nc.dram_tensor("hd_w1", [2, 128, 8, 512], BF16,
                                   kind="ExternalInput")
    dram["hd_b1"] = nc.dram_tensor("hd_b1", [2, 128, 4], F32,
                                   kind="ExternalInput")
    dram["hd_w2"] = nc.dram_tensor("hd_w2", [2, 128, 4, 2], F32,
                                   kind="ExternalInput")
    dram["hd_b2"] = nc.dram_tensor("hd_b2", [1, 2, 2], F32,
                                   kind="ExternalInput")
    out_logits = nc.dram_tensor("logits", [1, 4], F32, kind="ExternalOutput")

    rg_pairs = [[0, 1], [2, 3], [4, 5], [6, 7]]

    with tile.TileContext(nc) as tc:
        with (
            nc.allow_low_precision(
                reason="deliberate fp8/bf16 activation pipeline"),
            tc.tile_pool(name="act", bufs=1) as act,
            tc.tile_pool(name="w", bufs=3) as wpool,
            tc.tile_pool(name="vext", bufs=1) as vpool,
            tc.tile_pool(name="consts", bufs=1) as cpool,
            tc.tile_pool(name="x", bufs=3) as xpool,
            tc.tile_pool(name="f8", bufs=3) as f8pool,
            tc.tile_pool(name="pbig", bufs=2, space="PSUM") as pbig,
            tc.tile_pool(name="pctx", bufs=2, space="PSUM") as pctx,
            tc.tile_pool(name="dram", bufs=1, space="DRAM") as dpool,
        ):
            # ---- constants ----
            ones2 = cpool.tile([1, 2, 512], FP8, tag="ones2")
            nc.vector.memset(ones2[:], 1.0)
            eps_col = cpool.tile([128, 1], F32, tag="eps_col")
            nc.vector.memset(eps_col[:], EPS)
            zero_col = cpool.tile([128, 1], F32, tag="zero_col")
            nc.vector.memset(zero_col[:], 0.0)
            oavg_bf = cpool.tile([128, 128], BF16, tag="oavg_bf")
            nc.vector.memset(oavg_bf[:], 1.0 / D)
            lng_sb = cpool.tile([128, LAYERS, 3, 4], F32, tag="lng")
            nc.sync.dma_start(lng_sb[:], dram["ln_g"][:])
            lnb_sb = cpool.tile([128, LAYERS, 3, 4], F32, tag="lnb")
            nc.sync.dma_start(lnb_sb[:], dram["ln_b"][:])
            consts = dict(ones2=ones2, oavg_bf=oavg_bf, eps_col=eps_col,
                          zero_col=zero_col)
            pools = dict(act=act, w=wpool, vext=vpool, consts=consts,
                         pbig=pbig, pctx=pctx, xpool=xpool,
                         f8pool=f8pool,
                         lng_sb=[lng_sb[:, li] for li in range(LAYERS)],
                         lnb_sb=[lnb_sb[:, li] for li in range(LAYERS)])

            vts = []
            for st in range(KT8):
                vt = vpool.tile([128, H, DK + 64], BF16, tag=f"v{st}")
                nc.vector.memset(vt[:, :, DK:DK + 64], 1.0)
                vts.append(vt)
            pools["vts"] = vts

            pid = nc.sync.partition_id()
            partner_par = 1 - (pid % 2)

            def start_exchange(li, nq, x_f8, x_part):
                """Stage + AllGather + readback for S-half nq; issued on the
                SP queue so it fires on data-readiness, not engine order."""
                s0 = nq * 512
                ag_in = dpool.tile([128, DC, 512], FP8, tag=f"agi{li}_{nq}")
                ag_out = dpool.tile([2, 128, DC, 512], FP8,
                                    tag=f"ago{li}_{nq}")
                nc.sync.dma_start(ag_in[:], x_f8[:, :, s0:s0 + 512])
                nc.gpsimd.collective_compute(
                    "AllGather", OP.bypass, replica_groups=rg_pairs,
                    ins=[ag_in.opt()], outs=[ag_out.opt()])
                nc.sync.dma_start(
                    x_part[:, :, s0:s0 + 512],
                    ag_out[ds(partner_par, 1), :, :, :].opt())

            # ---- layer 0 input projection: x0T = w_in^T @ wristT + b + peT
            peT_sb = xpool.tile([128, DC, S], BF16, tag="x")
            nc.sync.dma_start(peT_sb[:], dram["peT"][:])
            wT_sb = act.tile([IN, S], F32R, tag="wT")
            nc.sync.dma_start(wT_sb[:], dram["wT"][:])
            win_sb = act.tile([IN, D], F32R, tag="win")
            nc.sync.dma_start(win_sb[:], dram["w_in"][:])
            bin_sb = act.tile([128, DC], F32, tag="bin")
            nc.sync.dma_start(bin_sb[:], dram["b_in"][:])

            x_bf = xpool.tile([128, DC, 1024], BF16, tag="x")
            x_f8 = f8pool.tile([128, DC, 1024], FP8, tag="xf8", bufs=2)
            x_part = f8pool.tile([128, DC, 1024], FP8, tag="xpart", bufs=1)
            for nq in range(NQ):
                s0 = nq * 512
                for mc in range(DC):
                    ps = _pp(pools)
                    nc.tensor.matmul(ps,
                                     win_sb[:, mc * 128:(mc + 1) * 128],
                                     wT_sb[:, s0:s0 + 512],
                                     start=True, stop=True)
                    nc.vector.scalar_tensor_tensor(
                        out=x_bf[:, mc, s0:s0 + 512], in0=ps,
                        scalar=bin_sb[:, mc:mc + 1],
                        in1=peT_sb[:, mc, s0:s0 + 512],
                        op0=OP.add, op1=OP.add)
                    eng = nc.gpsimd if mc % 2 == 0 else nc.vector
                    eng.tensor_copy(out=x_f8[:, mc, s0:s0 + 512],
                                    in_=x_bf[:, mc, s0:s0 + 512])
                start_exchange(0, nq, x_f8, x_part)

            for li in range(n_layers):
                lw = li % LAYERS
                xc_bf, xc_f8 = _emit_attn(nc, pools, dram, x_bf, x_f8,
                                          x_part, lw, 0)
                xs_bf, xs_f8 = _emit_attn(nc, pools, dram, xc_bf, xc_f8,
                                          xc_f8, lw, 1)
                last = li == n_layers - 1
                post_half = None
                if not last:
                    x_part = f8pool.tile([128, DC, 1024], FP8, tag="xpart",
                                         bufs=1)

                    def post_half(nq, xbf_new, xf8_new, _li=li + 1,
                                  _xp=x_part):
                        start_exchange(_li, nq, xf8_new, _xp)

                x_bf, x_f8 = _emit_ffn(nc, pools, dram, xs_bf, xs_f8, lw,
                                       post_half=post_half)

            # ---- mean pool over S -> pairwise allgather -> heads ----
            mh = act.tile([128, DC, 2], F32, tag="meanh")
            for nq in range(NQ):
                for dc in range(DC):
                    nc.vector.tensor_reduce(out=mh[:, dc, nq:nq + 1],
                                            in_=x_bf[:, dc,
                                                     nq * 512:(nq + 1) * 512],
                                            axis=mybir.AxisListType.X,
                                            op=OP.add)
            mean_sb = act.tile([128, DC, 1], F32, tag="mean")
            for dc in range(DC):
                nc.vector.tensor_tensor(out=mean_sb[:, dc, :],
                                        in0=mh[:, dc, 0:1],
                                        in1=mh[:, dc, 1:2], op=OP.add)
            mb_in = dpool.tile([DC, 128, 1], F32, tag="mbin")
            for dc in range(DC):
                nc.sync.dma_start(mb_in[dc], mean_sb[:, dc, :])
            mb_out = dpool.tile([2 * DC, 128, 1], F32, tag="mbout")
            nc.gpsimd.collective_compute(
                "AllGather", OP.bypass, replica_groups=rg_pairs,
                ins=[mb_in.opt()], outs=[mb_out.opt()])
            fusedF = act.tile([128, 2 * DC, 1], F32, tag="fusedF")
            for kc in range(2 * DC):
                nc.sync.dma_start(fusedF[:, kc, :], mb_out[kc])
            fusedT = act.tile([128, 2 * DC, 1], BF16, tag="fusedT")
            nc.vector.tensor_copy(out=fusedT[:], in_=fusedF[:])

            hb2 = act.tile([1, 2, 2], F32, tag="hb2")
            nc.sync.dma_start(hb2[:], dram["hd_b2"][:])
            logits_sb = act.tile([1, 4], F32, tag="logits")
            for hd in range(2):
                hw1 = act.tile([128, 8, 512], BF16, tag="hT", bufs=1)
                nc.sync.dma_start(hw1[:], dram["hd_w1"][hd])
                hw2 = act.tile([128, 4, 2], F32, tag="hw2", bufs=2)
                nc.sync.dma_start(hw2[:], dram["hd_w2"][hd])
                hb1 = act.tile([128, 4], F32, tag="hb1", bufs=2)
                nc.sync.dma_start(hb1[:], dram["hd_b1"][hd])
                o1 = act.tile([128, 4, 1], F32, tag="o1", bufs=2)
                for mc in range(DC):
                    ps = _pp(pools)
                    for kc in range(2 * DC):
                        nc.tensor.matmul(
                            ps[:, 0:1],
                            hw1[:, kc, mc * 128:(mc + 1) * 128],
                            fusedT[:, kc, :],
                            start=(kc == 0), stop=(kc == 2 * DC - 1))
                    nc.vector.tensor_scalar(out=o1[:, mc, :], in0=ps[:, 0:1],
                                            scalar1=hb1[:, mc:mc + 1],
                                            scalar2=0.0, op0=OP.add,
                                            op1=OP.max)
                lp = _pp(pools)
                for kc in range(DC):
                    nc.tensor.matmul(lp[0:1, 0:2], o1[:, kc, :],
                                     hw2[:, kc, :],
                                     start=(kc == 0), stop=(kc == DC - 1))
                nc.vector.tensor_tensor(out=logits_sb[0:1, hd * 2:hd * 2 + 2],
                                        in0=lp[0:1, 0:2], in1=hb2[0:1, hd, :],
                                        op=OP.add)
            nc.sync.dma_start(out_logits[:], logits_sb[:])

    nc.compile()
    return nc


def _prep(inputs):
    f32 = np.float32

    def g(k):
        return np.asarray(inputs[k], f32)

    lw, rw = g("left_wrist"), g("right_wrist")
    Wl, bl, Wr, br, pe = g("Wl"), g("bl"), g("Wr"), g("br"), g("pe")
    mha_w, mha_b = g("mha_w"), g("mha_b")
    mha_ln_g, mha_ln_b = g("mha_ln_g"), g("mha_ln_b")
    ff_w1, ff_b1, ff_w2, ff_b2 = g("ff_w1"), g("ff_b1"), g("ff_w2"), g("ff_b2")
    ff_ln_g, ff_ln_b = g("ff_ln_g"), g("ff_ln_b")
    h_w1 = [g("h1_w1"), g("h2_w1")]
    h_b1 = [g("h1_b1"), g("h2_b1")]
    h_w2 = [g("h1_w2"), g("h2_w2")]
    h_b2 = [g("h1_b2"), g("h2_b2")]

    peT = np.ascontiguousarray(
        pe.T.reshape(DC, 128, S).transpose(1, 0, 2)).astype(BF)

    per_ch = {}
    for ch in range(2):
        blocks = (0, 2) if ch == 0 else (1, 3)
        qkv = np.zeros((LAYERS, 2, 128, 16, 512), F8)
        qkb = np.zeros((LAYERS, 2, 128, 2, 4), f32)
        vb = np.zeros((LAYERS, 2, 1, 512), BF)
        obr = np.zeros((LAYERS, 2, 1, 2, 512), F8)
        lng = np.zeros((128, LAYERS, 3, 4), f32)
        lnb = np.zeros((128, LAYERS, 3, 4), f32)
        fw1 = np.zeros((LAYERS, 128, 4, 2048), F8)
        fb1r = np.zeros((LAYERS, 1, 2, 2048), F8)
        fw2 = np.zeros((LAYERS, 128, 16, 512), F8)
        fb2r = np.zeros((LAYERS, 1, 2, 512), F8)
        for li in range(LAYERS):
            for bi, blk in enumerate(blocks):
                for pi in range(3):  # q, k, v
                    qkv[li, bi, :, pi * 4:(pi + 1) * 4, :] = \
                        (mha_w[li, blk, pi] * WS).reshape(DC, 128, D) \
                        .transpose(1, 0, 2).astype(F8)
                qkv[li, bi, :, 12:16, :] = (mha_w[li, blk, 3] * WS) \
                    .reshape(DC, 128, D).transpose(1, 0, 2).astype(F8)
                for ci, pi in enumerate((0, 1)):  # q, k biases (unscaled)
                    qkb[li, bi, :, ci, :] = \
                        mha_b[li, blk, pi].reshape(DC, 128).T
                vb[li, bi, 0] = mha_b[li, blk, 2].astype(BF)
                obr[li, bi, 0, 0] = (mha_b[li, blk, 3] * WS).astype(F8)
                lng[:, li, bi, :] = mha_ln_g[li, blk].reshape(DC, 128).T
                lnb[:, li, bi, :] = mha_ln_b[li, blk].reshape(DC, 128).T
            lng[:, li, 2, :] = ff_ln_g[li, ch].reshape(DC, 128).T
            lnb[:, li, 2, :] = ff_ln_b[li, ch].reshape(DC, 128).T
            fw1[li] = (ff_w1[li, ch] * WS).reshape(DC, 128, F) \
                .transpose(1, 0, 2).astype(F8)
            fb1r[li, 0, 0] = (ff_b1[li, ch] * WS).astype(F8)
            fw2[li] = (ff_w2[li, ch] * WS).reshape(FC, 128, D) \
                .transpose(1, 0, 2).astype(F8)
            fb2r[li, 0, 0] = (ff_b2[li, ch] * WS).astype(F8)
        per_ch[ch] = dict(qkv_w=qkv, qk_b=qkb, v_b=vb, o_b=obr,
                          ln_g=lng, ln_b=lnb, ff_w1=fw1, ff_b1r=fb1r,
                          ff_w2=fw2, ff_b2r=fb2r)

    hd_w1 = np.stack([(w / float(S)).reshape(2 * DC, 128, D)
                      .transpose(1, 0, 2) for w in h_w1]).astype(BF)
    hd_b1 = np.stack([b.reshape(DC, 128).T for b in h_b1]).astype(f32)
    hd_w2 = np.stack([w.reshape(DC, 128, 2).transpose(1, 0, 2)
                      for w in h_w2]).astype(f32)
    hd_b2 = np.stack([b.reshape(1, 2) for b in h_b2]).transpose(1, 0, 2).astype(f32)

    in_maps = []
    for core in range(NCORES):
        b, ch = core // 2, core % 2
        wrist = lw[b] if ch == 0 else rw[b]
        w_in = Wl if ch == 0 else Wr
        b_in = (bl if ch == 0 else br).reshape(DC, 128).T
        m = {k: np.ascontiguousarray(v) for k, v in per_ch[ch].items()}
        m["wT"] = np.ascontiguousarray(wrist.T)
        m["w_in"] = np.ascontiguousarray(w_in)
        m["b_in"] = np.ascontiguousarray(b_in.astype(f32))
        m["peT"] = peT
        m["hd_w1"] = hd_w1
        m["hd_b1"] = hd_b1
        m["hd_w2"] = hd_w2
        m["hd_b2"] = hd_b2
        in_maps.append(m)
    return in_maps


def run(inputs, trace=False, n_layers=LAYERS):
    key = ("nc", n_layers)
    if key not in _CACHE:
        _CACHE[key] = _build(n_layers)
    nc = _CACHE[key]
    in_maps = _prep(inputs)
    res = run_bass_kernel_spmd(nc, in_maps, core_ids=list(range(NCORES)),
                               trace=trace)
    logits1 = np.zeros((B, 2), np.float32)
    logits2 = np.zeros((B, 2), np.float32)
    for b in range(B):
        out = res.results[2 * b]["logits"]
        logits1[b] = out[0, 0:2]
        logits2[b] = out[0, 2:4]
    return (logits1, logits2), res


def kernel(**inputs):
    out, _ = run(inputs, trace=False)
    return out
